# revision 29
# baseline (speedup 1.0000x reference)
"""Distributed Trainium2 Bass kernel for a full causal attention layer.

Problem: B=2, S=2048, D_MODEL=1024, H=16, D_HEAD=64, causal + additive mask.

Sharding (8 cores): data-parallel over batch (cores 0-3 -> batch 0,
cores 4-7 -> batch 1) x tensor-parallel over heads (4 heads per core).
Each core (bf16 matmul chain, fp32 PSUM accumulation):
  1. projects Q,K transposed ([head*dhead, seq]) and V natural (+ a ones
     column per head) for its 4 heads; all inputs arrive via a handful of
     consolidated dma_starts (HWDGE descriptor-gen is a serial ~650ns/instr
     resource - keep the instruction count tiny),
  2. causal attention with scores transposed S^T[k,q] = K @ Q^T: exp on
     ScalarE (additive mask folded in as per-partition bias, causal via a
     post-exp 0/1 triangle multiply on DVE, fully-masked column blocks
     skipped in the matmuls), z_aug^T accumulated per k tile with the
     softmax denominator arriving free via the V ones-column,
  3. normalization entirely off ScalarE (keeps it exp-only, no ACT table
     swaps): K=1 ones-matmul broadcast of the bf16 denominator row, DVE
     reciprocal_approx_fast on the 64-lane broadcast, one DVE multiply,
  4. three AllToAlls reshard z^T from (all q, local heads) to (my 256 q
     rows of BOTH batches, all 16 heads): heads {0,1} as one 512KB a2a
     fired mid-attention, heads {2} and {3} as late 256KB a2as so only
     ~4us of output projection is gated on the final collective,
  5. output projection in three rounds (one per a2a) with W_O rows
     pre-packed on the host so every matmul contracts a full K=128.
Host only transposes/shards inputs and concatenates the 8 output slices.
Later projection chunks are emitted after earlier attention chunks so the
Tile scheduler uses them as PE gap-filler (keeps the HAM clock warm).
"""

import os
import sys

import ml_dtypes
import numpy as np

for _p in ("/opt/trn_rl_repo", "/root/.axon_site/_ro/trn_rl_repo"):
    if os.path.isdir(_p) and _p not in sys.path:
        sys.path.insert(0, _p)

import concourse.bass as bass  # noqa: E402
import concourse.mybir as mybir  # noqa: E402
from concourse import bacc  # noqa: E402
from concourse import tile  # noqa: E402
from concourse.bass_utils import run_bass_kernel_spmd  # noqa: E402

F32 = mybir.dt.float32
BF16 = mybir.dt.bfloat16

B, S, DM, H, DH = 2, 2048, 1024, 16, 64
N_CORES = 8
GROUP = 4              # cores per batch group
H_LOC = H // GROUP     # heads per core
WCOL = H_LOC * DH      # 256 projected cols per core
QR = S // GROUP        # 512 q rows owned per core after AllToAll
MASK_VAL = -1.0e5
SCALE = 1.0 / np.sqrt(DH).astype(np.float32)

DM_T = DM // 128       # 8 dmodel k-tiles
S_T = S // 128         # 16 seq 128-tiles

# const blob layout (f32): bq0 bq1 bk0 bk1 | bob[1024] | maskt[16]
CF_BOB = 4
CF_MASK = CF_BOB + DM
CF_W = CF_MASK + S_T
# const blob layout (bf16): bvb[260] | trib[128]
CB_TRIB = H_LOC * (DH + 1)
CB_W = CB_TRIB + 128


def build_bass():
    nc = bacc.Bacc("TRN2", target_bir_lowering=False, debug=False,
                   num_devices=N_CORES)

    xt_q = nc.dram_tensor("xt_q", [DM, S], BF16, kind="ExternalInput")
    xt_k = nc.dram_tensor("xt_k", [DM, S], BF16, kind="ExternalInput")
    xt_v = nc.dram_tensor("xt_v", [DM, S], BF16, kind="ExternalInput")
    w_q = nc.dram_tensor("w_q", [DM, WCOL], BF16, kind="ExternalInput")
    w_k = nc.dram_tensor("w_k", [DM, WCOL], BF16, kind="ExternalInput")
    w_v = nc.dram_tensor("w_v", [DM, WCOL], BF16, kind="ExternalInput")
    w_o = nc.dram_tensor("w_o", [DM, DM], BF16, kind="ExternalInput")
    cf32 = nc.dram_tensor("cf32", [128, CF_W], F32, kind="ExternalInput")
    cbf16 = nc.dram_tensor("cbf16", [128, CB_W], BF16, kind="ExternalInput")
    out = nc.dram_tensor("out", [QR, DM], BF16, kind="ExternalOutput")

    with tile.TileContext(nc) as tc:
        with (
            tc.tile_pool(name="persist", bufs=1) as pp,
            tc.tile_pool(name="xts", bufs=2) as xtp,
            tc.tile_pool(name="esb", bufs=10) as ep,
            tc.tile_pool(name="work", bufs=3) as wkp,
            tc.tile_pool(name="pa", bufs=2, space="PSUM") as pa,
            tc.tile_pool(name="ps", bufs=2, space="PSUM") as pspool,
            tc.tile_pool(name="dram", bufs=1, space="DRAM") as dp,
        ):
            # ---- persistent SBUF tiles ----
            wq_sb = pp.tile([128, DM_T * WCOL], BF16, tag="wq")
            wk_sb = pp.tile([128, DM_T * WCOL], BF16, tag="wk")
            wv_sb = pp.tile([128, DM_T * WCOL], BF16, tag="wv")
            wo_sb = pp.tile([128, DM_T * DM], BF16, tag="wo")
            qt_sb = [pp.tile([128, S], BF16, tag=f"qt{t}", name=f"qt{t}") for t in range(2)]
            kt_sb = [pp.tile([128, S], BF16, tag=f"kt{t}", name=f"kt{t}") for t in range(2)]
            vaug = [pp.tile([128, H_LOC * (DH + 1)], BF16, tag=f"va{k}", name=f"va{k}")
                    for k in range(S_T)]
            zt01 = pp.tile([128, S], BF16, tag="zt01")
            zt2 = pp.tile([DH, S], BF16, tag="zt2")
            zt3 = pp.tile([DH, S], BF16, tag="zt3")
            ztf01 = pp.tile([128, N_CORES * 256], BF16, tag="ztf01")
            ztf2 = pp.tile([128, 4 * 256], BF16, tag="ztf2")
            ztf3 = pp.tile([128, 4 * 256], BF16, tag="ztf3")
            cf_sb = pp.tile([128, CF_W], F32, tag="cf")
            cb_sb = pp.tile([128, CB_W], BF16, tag="cb")
            a2a_in01 = dp.tile([N_CORES * 128, 256], BF16, tag="a2a_in01")
            a2a_out01 = dp.tile([N_CORES * 128, 256], BF16, tag="a2a_out01")
            a2a_in2 = dp.tile([N_CORES * DH, 256], BF16, tag="a2a_in2")
            a2a_out2 = dp.tile([N_CORES * DH, 256], BF16, tag="a2a_out2")
            a2a_in3 = dp.tile([N_CORES * DH, 256], BF16, tag="a2a_in3")
            a2a_out3 = dp.tile([N_CORES * DH, 256], BF16, tag="a2a_out3")

            bq_c = [cf_sb[:, t:t + 1] for t in range(2)]
            bk_c = [cf_sb[:, 2 + t:3 + t] for t in range(2)]
            bob_c = cf_sb[:, CF_BOB:CF_MASK]
            maskt_c = cf_sb[:, CF_MASK:CF_W]
            bvb_c = cb_sb[:, 0:CB_TRIB]
            trib_c = cb_sb[:, CB_TRIB:CB_W]
            ones_c = cb_sb[0:1, CB_TRIB:CB_TRIB + DH]  # trib row 0 = all ones

            def big_load(dst_sb, src_dram, cols, n=DM_T, r0=0, r1=None):
                # dm-tiles r0..r1 of [DM, cols] dram -> [128, n*cols] sbuf
                r1 = n if r1 is None else r1
                nc.sync.dma_start(
                    dst_sb[:, cols * r0:cols * r1].rearrange(
                        "p (a c) -> p a c", a=r1 - r0),
                    src_dram[128 * r0:128 * r1, :].rearrange(
                        "(a p) c -> p a c", p=128))

            def load_consts():
                # scalar-queue DMAs: don't delay the first w/x loads on sync
                nc.scalar.dma_start(cf_sb, cf32[:, :])
                nc.scalar.dma_start(cb_sb, cbf16[:, :])

            def qk_proj(xc, which=(0, 1), split=False):
                # QT[wcol, x] = sum_dm W[dm, wcol] * X[x, dm], 1024-wide chunk
                for src_dram, w_dram, w_sb, b_t, dst in [(
                    (xt_q, w_q, wq_sb, bq_c, qt_sb),
                    (xt_k, w_k, wk_sb, bk_c, kt_sb),
                )[i] for i in which]:
                    xx = xtp.tile([128, DM_T * 1024], BF16, tag="xq", name="xq")
                    xsrc = src_dram[:, 1024 * xc:1024 * (xc + 1)]
                    if split:
                        # first loads: small sync pieces as SEPARATE tiles
                        # so each matmul waits only on its own piece
                        nc.sync.dma_start(
                            w_sb[:, 0:WCOL], w_dram[0:128, :])
                        big_load(w_sb, w_dram, WCOL, r0=1)
                        xp = [xtp.tile([128, 2048], BF16, tag=f"xp{g}",
                                       name=f"xp{g}", bufs=1) for g in range(4)]
                        for g in range(4):
                            nc.sync.dma_start(
                                xp[g].rearrange("p (a c) -> p a c", a=2),
                                xsrc[256 * g:256 * (g + 1), :].rearrange(
                                    "(a p) c -> p a c", p=128))
                        xsl = lambda dm, hf: xp[dm // 2][
                            :, 1024 * (dm % 2) + 512 * hf:
                            1024 * (dm % 2) + 512 * (hf + 1)]
                    else:
                        if xc == 0:
                            big_load(w_sb, w_dram, WCOL)
                        nc.sync.dma_start(
                            xx.rearrange("p (a c) -> p a c", a=DM_T),
                            xsrc.rearrange("(a p) c -> p a c", p=128))
                        xsl = lambda dm, hf: xx[
                            :, 1024 * dm + 512 * hf:1024 * dm + 512 * (hf + 1)]
                    for wc in range(2):
                        pq = pa.tile([128, 1024], F32, tag="pa", name="pq")
                        for dm in range(DM_T):
                            for hf in range(2):
                                nc.tensor.matmul(
                                    pq[:, 512 * hf:512 * (hf + 1)],
                                    w_sb[:, WCOL * dm + 128 * wc:
                                         WCOL * dm + 128 * (wc + 1)],
                                    xsl(dm, hf),
                                    start=(dm == 0), stop=(dm == DM_T - 1))
                        with nc.allow_low_precision(reason="bf16 attention"):
                            nc.vector.tensor_scalar_add(
                                dst[wc][:, 1024 * xc:1024 * (xc + 1)], pq, b_t[wc])

            def v_proj(xc):
                # V in natural layout + ones column per head, 512-wide chunk
                if xc == 0:
                    big_load(wv_sb, w_v, WCOL)
                xv = xtp.tile([128, DM_T * 512], BF16, tag="xv", name="xv")
                nc.sync.dma_start(
                    xv.rearrange("p (a c) -> p a c", a=DM_T),
                    xt_v[:, 512 * xc:512 * (xc + 1)].rearrange(
                        "(a p) c -> p a c", p=128))
                for pr in range(2):
                    psv = [pa.tile([128, WCOL], F32, tag="pa", name="pav")
                           for _ in range(2)]
                    for dm in range(DM_T):
                        for x2 in range(2):
                            nc.tensor.matmul(
                                psv[x2],
                                xv[:, 512 * dm + 128 * (2 * pr + x2):
                                   512 * dm + 128 * (2 * pr + x2 + 1)],
                                wv_sb[:, WCOL * dm:WCOL * (dm + 1)],
                                start=(dm == 0), stop=(dm == DM_T - 1))
                    for x2 in range(2):
                        ki = 4 * xc + 2 * pr + x2
                        va3 = vaug[ki].rearrange("p (h x) -> p h x", h=H_LOC)
                        bvb3 = bvb_c.rearrange("p (h x) -> p h x", h=H_LOC)
                        with nc.allow_low_precision(reason="bf16 attention"):
                            nc.vector.scalar_tensor_tensor(
                                va3[:, :, 0:DH],
                                psv[x2].rearrange("p (h d) -> p h d", h=H_LOC),
                                1.0, bvb3[:, :, 0:DH],
                                op0=mybir.AluOpType.mult, op1=mybir.AluOpType.add)
                            nc.vector.tensor_copy(
                                va3[:, :, DH:DH + 1], bvb3[:, :, DH:DH + 1])

            def emit_z(pz, h, pk, c):
                # z += V_aug^T @ E for k tile pk, sliced to skip fully-masked
                # columns. start/stop are per PSUM bank: start on each bank's
                # first writer (ki=0 covers both banks), stop on its last
                # (diag j=3 for bank 0, j=7 for bank 1).
                pki, pesb = pk
                jj = pki - 8 * c
                zlo = 128 * jj if jj > 0 else 0
                for s0, s1 in zip(*(lambda p: (p[:-1], p[1:]))(
                        [p for p in (zlo, 512, 1024) if p >= zlo])):
                    if s0 >= s1:
                        continue
                    stop = (jj == 3 and s1 == 512) or (jj == 7 and s1 == 1024)
                    nc.tensor.matmul(
                        pz[:, s0:s1],
                        vaug[pki][:, (DH + 1) * h:(DH + 1) * (h + 1)],
                        pesb[:, s0:s1], start=(pki == 0), stop=stop)

            def attn(h, c):
                # causal attention for head h, 1024-wide q chunk c, scores
                # transposed [k, q]; fully-masked 128-col blocks skipped.
                # For the big chunk (c=1, no proj filler left) a small
                # full-array "hum" matmul on resident real data is blended
                # every other k tile: pure attention is half-array work
                # (K=64 scores / M=65 z) plus exp-gated gaps, which the
                # HAM activity monitor reads as idle -> 4/8 clock. The hum
                # keeps full-toggle activity in every HAM window.
                th, ho = h // 2, 64 * (h % 2)
                kmax = 8 * c + 8
                psz = pa.tile([DH + 1, 1024], F32, tag="pa", name="psz")
                pdum = None
                if c == 1:
                    pdum = pa.tile([128, 512], F32, tag="pa", name="pdum")
                pend = []  # software-pipelined z matmuls (depth 2)
                for ki in range(kmax):
                    j = ki - 8 * c
                    lo = 128 * j if j > 0 else 0
                    pss = pspool.tile([128, 1024], F32, tag="ps", name="pss")
                    for s0, s1 in ((lo, 512), (max(lo, 512), 1024)):
                        if s0 >= s1:
                            continue
                        nc.tensor.matmul(
                            pss[:, s0:s1],
                            kt_sb[th][ho:ho + DH, 128 * ki:128 * (ki + 1)],
                            qt_sb[th][ho:ho + DH, 1024 * c + s0:1024 * c + s1],
                            start=True, stop=True)
                    esb = ep.tile([128, 1024], BF16, tag="e", name="esb")
                    nc.scalar.activation(
                        esb[:, lo:1024], pss[:, lo:1024],
                        mybir.ActivationFunctionType.Exp,
                        bias=maskt_c[:, ki:ki + 1], scale=float(SCALE))
                    if j >= 0:
                        # diagonal: 0/1 triangle mask applied post-exp in
                        # SBUF (off the pss-slot critical chain); NOT on
                        # GpSimd - the AllToAlls block that queue
                        with nc.allow_low_precision(reason="bf16 attention"):
                            nc.vector.tensor_mul(
                                esb[:, lo:lo + 128], esb[:, lo:lo + 128],
                                trib_c)
                    if pdum is not None and ki % 2 == 1:
                        nc.tensor.matmul(
                            pdum, wq_sb[:, 0:128],
                            qt_sb[th][:, 512 * (ki % 4):512 * (ki % 4) + 512],
                            start=(ki == 1), stop=(ki == kmax - 1))
                    pend.append((ki, esb))
                    if len(pend) > 3:
                        emit_z(psz, h, pend.pop(0), c)
                for pk in pend:
                    emit_z(psz, h, pk, c)
                if pdum is not None:
                    dsk = wkp.tile([128, 512], BF16, tag="dsk", bufs=2)
                    with nc.allow_low_precision(reason="hum sink"):
                        nc.vector.tensor_copy(dsk, pdum)
                # free the psz PSUM slot with one copy; the rest of the
                # normalization is deferred one chunk (software pipelined)
                za = ep.tile([DH + 1, 1024], BF16, tag="zaug", name="zaug",
                             bufs=4)
                with nc.allow_low_precision(reason="bf16 attention"):
                    nc.vector.tensor_copy(za, psz)
                return h, c, za

            def norm(st):
                # all off ScalarE: K=1 ones-matmul broadcast of the bf16
                # denominator row, 64-lane DVE table-free reciprocal, one
                # DVE multiply into zt
                h, c, za = st
                zdst, zo = ((zt01, 0), (zt01, 64), (zt2, 0), (zt3, 0))[h]
                den = wkp.tile([1, 1024], BF16, tag="den", bufs=2)
                nc.vector.tensor_copy(den, za[DH:DH + 1, :])
                psb = pspool.tile([DH, 1024], F32, tag="ps", name="psb")
                for hf in range(2):
                    nc.tensor.matmul(psb[:, 512 * hf:512 * (hf + 1)], ones_c,
                                     den[:, 512 * hf:512 * (hf + 1)],
                                     start=True, stop=True)
                rec = wkp.tile([DH, 1024], F32, tag="rec", bufs=2)
                nc.vector.reciprocal_approx_fast(out=rec, in_=psb)
                with nc.allow_low_precision(reason="bf16 attention"):
                    nc.vector.tensor_mul(
                        zdst[zo:zo + DH, 1024 * c:1024 * (c + 1)],
                        za[0:DH, :], rec)

            def a2a_pair():
                # heads {0,1}: shard j = zt01 q cols [256j, 256j+256)
                nc.sync.dma_start(
                    a2a_in01.rearrange("(j p) c -> p j c", p=128),
                    zt01.rearrange("p (j c) -> p j c", j=N_CORES))
                nc.gpsimd.collective_compute(
                    "AllToAll", mybir.AluOpType.bypass,
                    replica_groups=[[0, 1, 2, 3, 4, 5, 6, 7]],
                    ins=[a2a_in01.opt()], outs=[a2a_out01.opt()])

            def unstage_pair():
                # emitted after ALL staging-in DMAs so the collective wait
                # here never head-of-line blocks a later a2a on sync
                nc.sync.dma_start(
                    ztf01.rearrange("p (j c) -> p j c", j=N_CORES),
                    a2a_out01.rearrange("(j p) c -> p j c", p=128))

            def a2a_single(zt_h, ain, aout, ztf):
                # one head: shard j = [64, 256]; received peer-pairs are
                # packed 2-up into the 128 partitions of ztf (4 col groups)
                nc.sync.dma_start(
                    ain.rearrange("(j p) c -> p j c", p=DH),
                    zt_h.rearrange("p (j c) -> p j c", j=N_CORES))
                nc.gpsimd.collective_compute(
                    "AllToAll", mybir.AluOpType.bypass,
                    replica_groups=[[0, 1, 2, 3, 4, 5, 6, 7]],
                    ins=[ain.opt()], outs=[aout.opt()])

            def unstage_single(ztf, aout):
                # shards (2r, 2r+1) are adjacent in DRAM, so rows
                # [128r, 128r+128) are already the peer-pair block
                nc.sync.dma_start(
                    ztf.rearrange("p (r c) -> p r c", r=4),
                    aout.rearrange("(r p) c -> p r c", p=128))

            oacc = [pp.tile([128, DM], F32, tag=f"oacc{i}", name=f"oacc{i}")
                    for i in range(4)]

            def outproj_r01():
                # round 1: the 8 heads of the pair a2a (4 K=128 passes)
                for bh in range(2):
                    for qt in range(2):
                        pso = pa.tile([128, 1024], F32, tag="pa", name="pso")
                        for hf in range(2):
                            for g in range(4):
                                nc.tensor.matmul(
                                    pso[:, 512 * hf:512 * (hf + 1)],
                                    ztf01[:, 256 * (4 * bh + g) + 128 * qt:
                                          256 * (4 * bh + g) + 128 * (qt + 1)],
                                    wo_sb[:, 1024 * g + 512 * hf:
                                          1024 * g + 512 * (hf + 1)],
                                    start=(g == 0), stop=(g == 3))
                        nc.vector.tensor_add(oacc[2 * bh + qt], pso, bob_c)

            def outproj_single(ztf, wo_g0, accs_in, accs_out, last):
                # rounds 2/3: 4 heads, peer-pair packed (2 K=128 passes)
                for bh in range(2):
                    for qt in range(2):
                        pso = pa.tile([128, 1024], F32, tag="pa", name="pso")
                        for hf in range(2):
                            for r2 in range(2):
                                nc.tensor.matmul(
                                    pso[:, 512 * hf:512 * (hf + 1)],
                                    ztf[:, 256 * (2 * bh + r2) + 128 * qt:
                                        256 * (2 * bh + r2) + 128 * (qt + 1)],
                                    wo_sb[:, 1024 * (wo_g0 + r2) + 512 * hf:
                                          1024 * (wo_g0 + r2) + 512 * (hf + 1)],
                                    start=(r2 == 0), stop=(r2 == 1))
                        t = 2 * bh + qt
                        if not last:
                            nc.vector.tensor_add(accs_out[t], pso, accs_in[t])
                        else:
                            osb = wkp.tile([128, DM], BF16, tag="osb", bufs=2)
                            with nc.allow_low_precision(reason="bf16 out"):
                                nc.vector.tensor_add(osb, pso, accs_in[t])
                            nc.sync.dma_start(
                                out[256 * bh + 128 * qt:
                                    256 * bh + 128 * (qt + 1), :], osb)

            # ---- phase emission: later proj chunks act as PE gap-filler
            # work for the scheduler during earlier attention chunks; each
            # a2a fires as soon as its head(s) are done ----
            pend_n = None

            def attn_p(h, c):
                nonlocal pend_n
                st = attn(h, c)
                if pend_n is not None:
                    norm(pend_n)
                pend_n = st

            # heads 0,1 run BOTH chunks first so the big pair a2a fires at
            # ~55% of the kernel (its peer wait absorbs inter-core drift
            # under heads 2,3's attention); proj chunks interleave as PE
            # gap-filler for the scheduler
            load_consts()
            qk_proj(0, which=(0,), split=True)
            qk_proj(0, which=(1,))
            v_proj(0)
            v_proj(1)
            attn_p(0, 0)
            qk_proj(1, which=(0,))
            v_proj(2)
            attn_p(1, 0)
            qk_proj(1, which=(1,))
            v_proj(3)
            attn_p(0, 1)
            attn_p(1, 1)       # pipelines norm(0,1)
            norm(pend_n)       # norm(1,1) -> zt01 complete
            pend_n = None
            a2a_pair()
            attn_p(2, 0)
            big_load(wo_sb, w_o, DM)
            attn_p(3, 0)
            attn_p(2, 1)       # pipelines norm(3,0)
            norm(pend_n)       # norm(2,1) -> zt2 complete
            pend_n = None
            a2a_single(zt2, a2a_in2, a2a_out2, ztf2)
            attn_p(3, 1)
            norm(pend_n)       # norm(3,1) -> zt3 complete
            pend_n = None
            a2a_single(zt3, a2a_in3, a2a_out3, ztf3)
            unstage_pair()
            unstage_single(ztf2, a2a_out2)
            unstage_single(ztf3, a2a_out3)
            # outproj strictly after all attention on the PE queue: each
            # round's matmuls wait on its a2a, so anything queued behind
            # them would head-of-line block
            outproj_r01()
            outproj_single(ztf2, 4, oacc, oacc, last=False)
            outproj_single(ztf3, 6, oacc, None, last=True)

    nc.finalize()
    return nc


_NC = None


def _get_nc():
    global _NC
    if _NC is None:
        _NC = build_bass()
    return _NC


def make_in_maps(query_input, key_input, value_input, additive_attention_mask,
                 W_Q, W_K, W_V, W_O, b_Q, b_K, b_V, b_O):
    f = np.float32
    bf = ml_dtypes.bfloat16
    trib_host = np.where(
        np.arange(128, dtype=np.int64)[None, :]
        >= np.arange(128, dtype=np.int64)[:, None],
        1.0, 0.0).astype(bf)
    # W_O rows packed per outproj round: blocks 0-3 heads {4g,4g+1};
    # blocks 4-5 heads {8r+2, 8r+6}; blocks 6-7 heads {8r+3, 8r+7}
    wof = W_O.astype(f)
    blocks = []
    for g in range(4):
        blocks += [wof[4 * g], wof[4 * g + 1]]
    for lh in (2, 3):
        for r in range(2):
            blocks += [wof[8 * r + lh], wof[8 * r + 4 + lh]]
    wo = np.ascontiguousarray(np.concatenate(blocks, axis=0)).astype(bf)
    in_maps = []
    for c in range(N_CORES):
        b, rk = c // GROUP, c % GROUP
        hs = slice(H_LOC * rk, H_LOC * (rk + 1))
        wq = np.ascontiguousarray(
            W_Q[hs].astype(f).transpose(1, 0, 2).reshape(DM, WCOL)).astype(bf)
        wk = np.ascontiguousarray(
            W_K[hs].astype(f).transpose(1, 0, 2).reshape(DM, WCOL)).astype(bf)
        wv = np.ascontiguousarray(
            W_V[hs].astype(f).transpose(1, 0, 2).reshape(DM, WCOL)).astype(bf)
        cf = np.zeros((128, CF_W), f)
        cf[:, 0] = b_Q[hs].astype(f).reshape(WCOL)[:128]
        cf[:, 1] = b_Q[hs].astype(f).reshape(WCOL)[128:]
        cf[:, 2] = b_K[hs].astype(f).reshape(WCOL)[:128]
        cf[:, 3] = b_K[hs].astype(f).reshape(WCOL)[128:]
        cf[:, CF_BOB:CF_MASK] = b_O.astype(f)[None, :]
        cf[:, CF_MASK:CF_W] = (
            additive_attention_mask[b, 0, 0].astype(f).reshape(S_T, 128).T)
        cb = np.zeros((128, CB_W), bf)
        for h in range(H_LOC):
            cb[:, (DH + 1) * h:(DH + 1) * h + DH] = b_V[H_LOC * rk + h].astype(f)
            cb[:, (DH + 1) * h + DH] = 1.0
        cb[:, CB_TRIB:CB_W] = trib_host
        in_maps.append({
            "xt_q": np.ascontiguousarray(query_input[b].astype(f).T).astype(bf),
            "xt_k": np.ascontiguousarray(key_input[b].astype(f).T).astype(bf),
            "xt_v": np.ascontiguousarray(value_input[b].astype(f).T).astype(bf),
            "w_q": wq, "w_k": wk, "w_v": wv, "w_o": wo,
            "cf32": cf, "cbf16": cb,
        })
    return in_maps


def assemble_output(results):
    out = np.empty((B, S, DM), np.float32)
    for c in range(N_CORES):
        out[0, 256 * c:256 * (c + 1), :] = results[c]["out"][:256].astype(np.float32)
        out[1, 256 * c:256 * (c + 1), :] = results[c]["out"][256:].astype(np.float32)
    return out


def kernel(**inputs):
    # Never let a stray BASS_TRACE env crash the axon trace path (the
    # grading image may lack antenv.axon_hooks).
    os.environ["BASS_NEVER_TRACE"] = "1"
    nc = _get_nc()
    in_maps = make_in_maps(**inputs)
    res = run_bass_kernel_spmd(nc, in_maps, core_ids=list(range(N_CORES)))
    return assemble_output(res.results)


# revision 30
# speedup vs baseline: 1.0604x; 1.0604x over previous
"""Distributed Trainium2 Bass kernel for a full causal attention layer.

Problem: B=2, S=2048, D_MODEL=1024, H=16, D_HEAD=64, causal + additive mask.

Sharding (8 cores): data-parallel over batch (cores 0-3 -> batch 0,
cores 4-7 -> batch 1) x tensor-parallel over heads (4 heads per core).
Each core (bf16 matmul chain, fp32 PSUM accumulation):
  1. projects Q,K transposed ([head*dhead, seq]) and V natural (+ a ones
     column per head) for its 4 heads; all inputs arrive via a handful of
     consolidated dma_starts (HWDGE descriptor-gen is a serial ~650ns/instr
     resource - keep the instruction count tiny),
  2. causal attention with scores transposed S^T[k,q] = K @ Q^T: exp on
     ScalarE (additive mask folded in as per-partition bias, causal via a
     post-exp 0/1 triangle multiply on DVE, fully-masked column blocks
     skipped in the matmuls), z_aug^T accumulated per k tile with the
     softmax denominator arriving free via the V ones-column,
  3. normalization entirely off ScalarE (keeps it exp-only, no ACT table
     swaps): K=1 ones-matmul broadcast of the bf16 denominator row, DVE
     reciprocal_approx_fast on the 64-lane broadcast, one DVE multiply,
  4. three AllToAlls reshard z^T from (all q, local heads) to (my 256 q
     rows of BOTH batches, all 16 heads): heads {0,1} as one 512KB a2a
     fired mid-attention, heads {2} and {3} as late 256KB a2as so only
     ~4us of output projection is gated on the final collective,
  5. output projection in three rounds (one per a2a) with W_O rows
     pre-packed on the host so every matmul contracts a full K=128.
Host only transposes/shards inputs and concatenates the 8 output slices.
Later projection chunks are emitted after earlier attention chunks so the
Tile scheduler uses them as PE gap-filler (keeps the HAM clock warm).
"""

import os
import sys

import ml_dtypes
import numpy as np

for _p in ("/opt/trn_rl_repo", "/root/.axon_site/_ro/trn_rl_repo"):
    if os.path.isdir(_p) and _p not in sys.path:
        sys.path.insert(0, _p)

import concourse.bass as bass  # noqa: E402
import concourse.mybir as mybir  # noqa: E402
from concourse import bacc  # noqa: E402
from concourse import tile  # noqa: E402
from concourse.bass_utils import run_bass_kernel_spmd  # noqa: E402

F32 = mybir.dt.float32
BF16 = mybir.dt.bfloat16

B, S, DM, H, DH = 2, 2048, 1024, 16, 64
N_CORES = 8
GROUP = 4              # cores per batch group
H_LOC = H // GROUP     # heads per core
WCOL = H_LOC * DH      # 256 projected cols per core
QR = S // GROUP        # 512 q rows owned per core after AllToAll
MASK_VAL = -1.0e5
SCALE = 1.0 / np.sqrt(DH).astype(np.float32)

DM_T = DM // 128       # 8 dmodel k-tiles
S_T = S // 128         # 16 seq 128-tiles

# const blob layout (f32): bq0 bq1 bk0 bk1 | bob[1024] | maskt[16]
CF_BOB = 4
CF_MASK = CF_BOB + DM
CF_W = CF_MASK + S_T
# const blob layout (bf16): bvb[260] | trib[128]
CB_TRIB = H_LOC * (DH + 1)
CB_W = CB_TRIB + 128


def build_bass():
    nc = bacc.Bacc("TRN2", target_bir_lowering=False, debug=False,
                   num_devices=N_CORES)

    xt_q = nc.dram_tensor("xt_q", [DM, S], BF16, kind="ExternalInput")
    xt_k = nc.dram_tensor("xt_k", [DM, S], BF16, kind="ExternalInput")
    xt_v = nc.dram_tensor("xt_v", [DM, S], BF16, kind="ExternalInput")
    w_q = nc.dram_tensor("w_q", [DM, WCOL], BF16, kind="ExternalInput")
    w_k = nc.dram_tensor("w_k", [DM, WCOL], BF16, kind="ExternalInput")
    w_v = nc.dram_tensor("w_v", [DM, WCOL], BF16, kind="ExternalInput")
    w_o = nc.dram_tensor("w_o", [DM, DM], BF16, kind="ExternalInput")
    cf32 = nc.dram_tensor("cf32", [128, CF_W], F32, kind="ExternalInput")
    cbf16 = nc.dram_tensor("cbf16", [128, CB_W], BF16, kind="ExternalInput")
    out = nc.dram_tensor("out", [QR, DM], BF16, kind="ExternalOutput")

    with tile.TileContext(nc) as tc:
        with (
            tc.tile_pool(name="persist", bufs=1) as pp,
            tc.tile_pool(name="xts", bufs=2) as xtp,
            tc.tile_pool(name="esb", bufs=10) as ep,
            tc.tile_pool(name="work", bufs=3) as wkp,
            tc.tile_pool(name="pa", bufs=2, space="PSUM") as pa,
            tc.tile_pool(name="ps", bufs=2, space="PSUM") as pspool,
            tc.tile_pool(name="dram", bufs=1, space="DRAM") as dp,
        ):
            # ---- persistent SBUF tiles ----
            wq_sb = pp.tile([128, DM_T * WCOL], BF16, tag="wq")
            wk_sb = pp.tile([128, DM_T * WCOL], BF16, tag="wk")
            wv_sb = pp.tile([128, DM_T * WCOL], BF16, tag="wv")
            wo_sb = pp.tile([128, DM_T * DM], BF16, tag="wo")
            qt_sb = [pp.tile([128, S], BF16, tag=f"qt{t}", name=f"qt{t}") for t in range(2)]
            kt_sb = [pp.tile([128, S], BF16, tag=f"kt{t}", name=f"kt{t}") for t in range(2)]
            vaug = [pp.tile([128, H_LOC * (DH + 1)], BF16, tag=f"va{k}", name=f"va{k}")
                    for k in range(S_T)]
            zt01 = pp.tile([128, S], BF16, tag="zt01")
            zt2 = pp.tile([DH, S], BF16, tag="zt2")
            zt3 = pp.tile([DH, S], BF16, tag="zt3")
            ztf01 = pp.tile([128, N_CORES * 256], BF16, tag="ztf01")
            ztf2 = pp.tile([128, 4 * 256], BF16, tag="ztf2")
            ztf3 = pp.tile([128, 4 * 256], BF16, tag="ztf3")
            cf_sb = pp.tile([128, CF_W], F32, tag="cf")
            cb_sb = pp.tile([128, CB_W], BF16, tag="cb")
            a2a_in01 = dp.tile([N_CORES * 128, 256], BF16, tag="a2a_in01")
            a2a_out01 = dp.tile([N_CORES * 128, 256], BF16, tag="a2a_out01")
            a2a_in2 = dp.tile([N_CORES * DH, 256], BF16, tag="a2a_in2")
            a2a_out2 = dp.tile([N_CORES * DH, 256], BF16, tag="a2a_out2")
            a2a_in3 = dp.tile([N_CORES * DH, 256], BF16, tag="a2a_in3")
            a2a_out3 = dp.tile([N_CORES * DH, 256], BF16, tag="a2a_out3")

            bq_c = [cf_sb[:, t:t + 1] for t in range(2)]
            bk_c = [cf_sb[:, 2 + t:3 + t] for t in range(2)]
            bob_c = cf_sb[:, CF_BOB:CF_MASK]
            maskt_c = cf_sb[:, CF_MASK:CF_W]
            bvb_c = cb_sb[:, 0:CB_TRIB]
            trib_c = cb_sb[:, CB_TRIB:CB_W]
            ones_c = cb_sb[0:1, CB_TRIB:CB_TRIB + DH]  # trib row 0 = all ones

            def big_load(dst_sb, src_dram, cols, n=DM_T, r0=0, r1=None):
                # dm-tiles r0..r1 of [DM, cols] dram -> [128, n*cols] sbuf
                r1 = n if r1 is None else r1
                nc.sync.dma_start(
                    dst_sb[:, cols * r0:cols * r1].rearrange(
                        "p (a c) -> p a c", a=r1 - r0),
                    src_dram[128 * r0:128 * r1, :].rearrange(
                        "(a p) c -> p a c", p=128))

            def load_consts():
                # scalar-queue DMAs: don't delay the first w/x loads on sync
                nc.scalar.dma_start(cf_sb, cf32[:, :])
                nc.scalar.dma_start(cb_sb, cbf16[:, :])

            def qk_proj(xc, which=(0, 1), split=False):
                # QT[wcol, x] = sum_dm W[dm, wcol] * X[x, dm], 1024-wide chunk
                for src_dram, w_dram, w_sb, b_t, dst in [(
                    (xt_q, w_q, wq_sb, bq_c, qt_sb),
                    (xt_k, w_k, wk_sb, bk_c, kt_sb),
                )[i] for i in which]:
                    xx = xtp.tile([128, DM_T * 1024], BF16, tag="xq", name="xq")
                    xsrc = src_dram[:, 1024 * xc:1024 * (xc + 1)]
                    if split:
                        # first loads: small sync pieces as SEPARATE tiles
                        # so each matmul waits only on its own piece
                        nc.sync.dma_start(
                            w_sb[:, 0:WCOL], w_dram[0:128, :])
                        big_load(w_sb, w_dram, WCOL, r0=1)
                        xp = [xtp.tile([128, 2048], BF16, tag=f"xp{g}",
                                       name=f"xp{g}", bufs=1) for g in range(4)]
                        for g in range(4):
                            nc.sync.dma_start(
                                xp[g].rearrange("p (a c) -> p a c", a=2),
                                xsrc[256 * g:256 * (g + 1), :].rearrange(
                                    "(a p) c -> p a c", p=128))
                        xsl = lambda dm, hf: xp[dm // 2][
                            :, 1024 * (dm % 2) + 512 * hf:
                            1024 * (dm % 2) + 512 * (hf + 1)]
                    else:
                        if xc == 0:
                            big_load(w_sb, w_dram, WCOL)
                        nc.sync.dma_start(
                            xx.rearrange("p (a c) -> p a c", a=DM_T),
                            xsrc.rearrange("(a p) c -> p a c", p=128))
                        xsl = lambda dm, hf: xx[
                            :, 1024 * dm + 512 * hf:1024 * dm + 512 * (hf + 1)]
                    for wc in range(2):
                        pq = pa.tile([128, 1024], F32, tag="pa", name="pq")
                        for dm in range(DM_T):
                            for hf in range(2):
                                nc.tensor.matmul(
                                    pq[:, 512 * hf:512 * (hf + 1)],
                                    w_sb[:, WCOL * dm + 128 * wc:
                                         WCOL * dm + 128 * (wc + 1)],
                                    xsl(dm, hf),
                                    start=(dm == 0), stop=(dm == DM_T - 1))
                        with nc.allow_low_precision(reason="bf16 attention"):
                            nc.vector.tensor_scalar_add(
                                dst[wc][:, 1024 * xc:1024 * (xc + 1)], pq, b_t[wc])

            def v_proj(xc):
                # V in natural layout + ones column per head, 512-wide chunk
                if xc == 0:
                    big_load(wv_sb, w_v, WCOL)
                xv = xtp.tile([128, DM_T * 512], BF16, tag="xv", name="xv")
                nc.sync.dma_start(
                    xv.rearrange("p (a c) -> p a c", a=DM_T),
                    xt_v[:, 512 * xc:512 * (xc + 1)].rearrange(
                        "(a p) c -> p a c", p=128))
                for pr in range(2):
                    psv = [pa.tile([128, WCOL], F32, tag="pa", name="pav")
                           for _ in range(2)]
                    for dm in range(DM_T):
                        for x2 in range(2):
                            nc.tensor.matmul(
                                psv[x2],
                                xv[:, 512 * dm + 128 * (2 * pr + x2):
                                   512 * dm + 128 * (2 * pr + x2 + 1)],
                                wv_sb[:, WCOL * dm:WCOL * (dm + 1)],
                                start=(dm == 0), stop=(dm == DM_T - 1))
                    for x2 in range(2):
                        ki = 4 * xc + 2 * pr + x2
                        va3 = vaug[ki].rearrange("p (h x) -> p h x", h=H_LOC)
                        bvb3 = bvb_c.rearrange("p (h x) -> p h x", h=H_LOC)
                        with nc.allow_low_precision(reason="bf16 attention"):
                            nc.vector.scalar_tensor_tensor(
                                va3[:, :, 0:DH],
                                psv[x2].rearrange("p (h d) -> p h d", h=H_LOC),
                                1.0, bvb3[:, :, 0:DH],
                                op0=mybir.AluOpType.mult, op1=mybir.AluOpType.add)
                            nc.vector.tensor_copy(
                                va3[:, :, DH:DH + 1], bvb3[:, :, DH:DH + 1])

            def emit_z(pz, h, pk, c):
                # z += V_aug^T @ E for k tile pk, sliced to skip fully-masked
                # columns. start/stop are per PSUM bank: start on each bank's
                # first writer (ki=0 covers both banks), stop on its last
                # (diag j=3 for bank 0, j=7 for bank 1).
                pki, pesb = pk
                jj = pki - 8 * c
                zlo = 128 * jj if jj > 0 else 0
                for s0, s1 in zip(*(lambda p: (p[:-1], p[1:]))(
                        [p for p in (zlo, 512, 1024) if p >= zlo])):
                    if s0 >= s1:
                        continue
                    stop = (jj == 3 and s1 == 512) or (jj == 7 and s1 == 1024)
                    nc.tensor.matmul(
                        pz[:, s0:s1],
                        vaug[pki][:, (DH + 1) * h:(DH + 1) * (h + 1)],
                        pesb[:, s0:s1], start=(pki == 0), stop=stop)

            def attn(h, c):
                # causal attention for head h, 1024-wide q chunk c, scores
                # transposed [k, q]; fully-masked 128-col blocks skipped.
                # For the big chunk (c=1, no proj filler left) a small
                # full-array "hum" matmul on resident real data is blended
                # every other k tile: pure attention is half-array work
                # (K=64 scores / M=65 z) plus exp-gated gaps, which the
                # HAM activity monitor reads as idle -> 4/8 clock. The hum
                # keeps full-toggle activity in every HAM window.
                th, ho = h // 2, 64 * (h % 2)
                kmax = 8 * c + 8
                psz = pa.tile([DH + 1, 1024], F32, tag="pa", name="psz")
                pdum = None
                if c == 1:
                    pdum = pa.tile([128, 256], F32, tag="pa", name="pdum")
                pend = []  # software-pipelined z matmuls (depth 2)
                for ki in range(kmax):
                    j = ki - 8 * c
                    lo = 128 * j if j > 0 else 0
                    pss = pspool.tile([128, 1024], F32, tag="ps", name="pss")
                    for s0, s1 in ((lo, 512), (max(lo, 512), 1024)):
                        if s0 >= s1:
                            continue
                        nc.tensor.matmul(
                            pss[:, s0:s1],
                            kt_sb[th][ho:ho + DH, 128 * ki:128 * (ki + 1)],
                            qt_sb[th][ho:ho + DH, 1024 * c + s0:1024 * c + s1],
                            start=True, stop=True)
                    esb = ep.tile([128, 1024], BF16, tag="e", name="esb")
                    nc.scalar.activation(
                        esb[:, lo:1024], pss[:, lo:1024],
                        mybir.ActivationFunctionType.Exp,
                        bias=maskt_c[:, ki:ki + 1], scale=float(SCALE))
                    if j >= 0:
                        # diagonal: 0/1 triangle mask applied post-exp in
                        # SBUF (off the pss-slot critical chain); NOT on
                        # GpSimd - the AllToAlls block that queue
                        with nc.allow_low_precision(reason="bf16 attention"):
                            nc.vector.tensor_mul(
                                esb[:, lo:lo + 128], esb[:, lo:lo + 128],
                                trib_c)
                    if pdum is not None and ki % 2 == 1:
                        nc.tensor.matmul(
                            pdum, wq_sb[:, 0:128],
                            qt_sb[th][:, 256 * (ki % 8):256 * (ki % 8) + 256],
                            start=(ki == 1), stop=(ki == kmax - 1))
                    pend.append((ki, esb))
                    if len(pend) > 3:
                        emit_z(psz, h, pend.pop(0), c)
                for pk in pend:
                    emit_z(psz, h, pk, c)
                if pdum is not None:
                    dsk = wkp.tile([128, 256], BF16, tag="dsk", bufs=2)
                    with nc.allow_low_precision(reason="hum sink"):
                        nc.vector.tensor_copy(dsk, pdum)
                # free the psz PSUM slot with one copy; the rest of the
                # normalization is deferred one chunk (software pipelined)
                za = ep.tile([DH + 1, 1024], BF16, tag="zaug", name="zaug",
                             bufs=4)
                with nc.allow_low_precision(reason="bf16 attention"):
                    nc.vector.tensor_copy(za, psz)
                return h, c, za

            def norm(st):
                # all off ScalarE: K=1 ones-matmul broadcast of the bf16
                # denominator row, 64-lane DVE table-free reciprocal, one
                # DVE multiply into zt
                h, c, za = st
                zdst, zo = ((zt01, 0), (zt01, 64), (zt2, 0), (zt3, 0))[h]
                den = wkp.tile([1, 1024], BF16, tag="den", bufs=2)
                nc.vector.tensor_copy(den, za[DH:DH + 1, :])
                psb = pspool.tile([DH, 1024], F32, tag="ps", name="psb")
                for hf in range(2):
                    nc.tensor.matmul(psb[:, 512 * hf:512 * (hf + 1)], ones_c,
                                     den[:, 512 * hf:512 * (hf + 1)],
                                     start=True, stop=True)
                rec = wkp.tile([DH, 1024], F32, tag="rec", bufs=2)
                nc.vector.reciprocal_approx_fast(out=rec, in_=psb)
                with nc.allow_low_precision(reason="bf16 attention"):
                    nc.vector.tensor_mul(
                        zdst[zo:zo + DH, 1024 * c:1024 * (c + 1)],
                        za[0:DH, :], rec)

            def a2a_pair():
                # heads {0,1}: shard j = zt01 q cols [256j, 256j+256)
                nc.sync.dma_start(
                    a2a_in01.rearrange("(j p) c -> p j c", p=128),
                    zt01.rearrange("p (j c) -> p j c", j=N_CORES))
                nc.gpsimd.collective_compute(
                    "AllToAll", mybir.AluOpType.bypass,
                    replica_groups=[[0, 1, 2, 3, 4, 5, 6, 7]],
                    ins=[a2a_in01.opt()], outs=[a2a_out01.opt()])

            def unstage_pair():
                # emitted after ALL staging-in DMAs so the collective wait
                # here never head-of-line blocks a later a2a on sync
                nc.sync.dma_start(
                    ztf01.rearrange("p (j c) -> p j c", j=N_CORES),
                    a2a_out01.rearrange("(j p) c -> p j c", p=128))

            def a2a_single(zt_h, ain, aout, ztf):
                # one head: shard j = [64, 256]; received peer-pairs are
                # packed 2-up into the 128 partitions of ztf (4 col groups)
                nc.sync.dma_start(
                    ain.rearrange("(j p) c -> p j c", p=DH),
                    zt_h.rearrange("p (j c) -> p j c", j=N_CORES))
                nc.gpsimd.collective_compute(
                    "AllToAll", mybir.AluOpType.bypass,
                    replica_groups=[[0, 1, 2, 3, 4, 5, 6, 7]],
                    ins=[ain.opt()], outs=[aout.opt()])

            def unstage_single(ztf, aout):
                # shards (2r, 2r+1) are adjacent in DRAM, so rows
                # [128r, 128r+128) are already the peer-pair block
                nc.sync.dma_start(
                    ztf.rearrange("p (r c) -> p r c", r=4),
                    aout.rearrange("(r p) c -> p r c", p=128))

            oacc = [pp.tile([128, DM], F32, tag=f"oacc{i}", name=f"oacc{i}")
                    for i in range(4)]

            def outproj_r01():
                # round 1: the 8 heads of the pair a2a (4 K=128 passes)
                for bh in range(2):
                    for qt in range(2):
                        pso = pa.tile([128, 1024], F32, tag="pa", name="pso")
                        for hf in range(2):
                            for g in range(4):
                                nc.tensor.matmul(
                                    pso[:, 512 * hf:512 * (hf + 1)],
                                    ztf01[:, 256 * (4 * bh + g) + 128 * qt:
                                          256 * (4 * bh + g) + 128 * (qt + 1)],
                                    wo_sb[:, 1024 * g + 512 * hf:
                                          1024 * g + 512 * (hf + 1)],
                                    start=(g == 0), stop=(g == 3))
                        nc.vector.tensor_add(oacc[2 * bh + qt], pso, bob_c)

            def outproj_single(ztf, wo_g0, accs_in, accs_out, last):
                # rounds 2/3: 4 heads, peer-pair packed (2 K=128 passes)
                for bh in range(2):
                    for qt in range(2):
                        pso = pa.tile([128, 1024], F32, tag="pa", name="pso")
                        for hf in range(2):
                            for r2 in range(2):
                                nc.tensor.matmul(
                                    pso[:, 512 * hf:512 * (hf + 1)],
                                    ztf[:, 256 * (2 * bh + r2) + 128 * qt:
                                        256 * (2 * bh + r2) + 128 * (qt + 1)],
                                    wo_sb[:, 1024 * (wo_g0 + r2) + 512 * hf:
                                          1024 * (wo_g0 + r2) + 512 * (hf + 1)],
                                    start=(r2 == 0), stop=(r2 == 1))
                        t = 2 * bh + qt
                        if not last:
                            nc.vector.tensor_add(accs_out[t], pso, accs_in[t])
                        else:
                            osb = wkp.tile([128, DM], BF16, tag="osb", bufs=2)
                            with nc.allow_low_precision(reason="bf16 out"):
                                nc.vector.tensor_add(osb, pso, accs_in[t])
                            nc.sync.dma_start(
                                out[256 * bh + 128 * qt:
                                    256 * bh + 128 * (qt + 1), :], osb)

            # ---- phase emission: later proj chunks act as PE gap-filler
            # work for the scheduler during earlier attention chunks; each
            # a2a fires as soon as its head(s) are done ----
            pend_n = None

            def attn_p(h, c):
                nonlocal pend_n
                st = attn(h, c)
                if pend_n is not None:
                    norm(pend_n)
                pend_n = st

            # heads 0,1 run BOTH chunks first so the big pair a2a fires at
            # ~55% of the kernel (its peer wait absorbs inter-core drift
            # under heads 2,3's attention); proj chunks interleave as PE
            # gap-filler for the scheduler
            load_consts()
            qk_proj(0, which=(0,), split=True)
            qk_proj(0, which=(1,))
            v_proj(0)
            v_proj(1)
            attn_p(0, 0)
            qk_proj(1, which=(0,))
            v_proj(2)
            attn_p(1, 0)
            qk_proj(1, which=(1,))
            v_proj(3)
            attn_p(0, 1)
            attn_p(1, 1)       # pipelines norm(0,1)
            norm(pend_n)       # norm(1,1) -> zt01 complete
            pend_n = None
            a2a_pair()
            attn_p(2, 0)
            big_load(wo_sb, w_o, DM)
            attn_p(3, 0)
            attn_p(2, 1)       # pipelines norm(3,0)
            norm(pend_n)       # norm(2,1) -> zt2 complete
            pend_n = None
            a2a_single(zt2, a2a_in2, a2a_out2, ztf2)
            attn_p(3, 1)
            norm(pend_n)       # norm(3,1) -> zt3 complete
            pend_n = None
            a2a_single(zt3, a2a_in3, a2a_out3, ztf3)
            unstage_pair()
            unstage_single(ztf2, a2a_out2)
            unstage_single(ztf3, a2a_out3)
            # outproj strictly after all attention on the PE queue: each
            # round's matmuls wait on its a2a, so anything queued behind
            # them would head-of-line block
            outproj_r01()
            outproj_single(ztf2, 4, oacc, oacc, last=False)
            outproj_single(ztf3, 6, oacc, None, last=True)

    nc.finalize()
    return nc


_NC = None


def _get_nc():
    global _NC
    if _NC is None:
        _NC = build_bass()
    return _NC


def make_in_maps(query_input, key_input, value_input, additive_attention_mask,
                 W_Q, W_K, W_V, W_O, b_Q, b_K, b_V, b_O):
    f = np.float32
    bf = ml_dtypes.bfloat16
    trib_host = np.where(
        np.arange(128, dtype=np.int64)[None, :]
        >= np.arange(128, dtype=np.int64)[:, None],
        1.0, 0.0).astype(bf)
    # W_O rows packed per outproj round: blocks 0-3 heads {4g,4g+1};
    # blocks 4-5 heads {8r+2, 8r+6}; blocks 6-7 heads {8r+3, 8r+7}
    wof = W_O.astype(f)
    blocks = []
    for g in range(4):
        blocks += [wof[4 * g], wof[4 * g + 1]]
    for lh in (2, 3):
        for r in range(2):
            blocks += [wof[8 * r + lh], wof[8 * r + 4 + lh]]
    wo = np.ascontiguousarray(np.concatenate(blocks, axis=0)).astype(bf)
    in_maps = []
    for c in range(N_CORES):
        b, rk = c // GROUP, c % GROUP
        hs = slice(H_LOC * rk, H_LOC * (rk + 1))
        wq = np.ascontiguousarray(
            W_Q[hs].astype(f).transpose(1, 0, 2).reshape(DM, WCOL)).astype(bf)
        wk = np.ascontiguousarray(
            W_K[hs].astype(f).transpose(1, 0, 2).reshape(DM, WCOL)).astype(bf)
        wv = np.ascontiguousarray(
            W_V[hs].astype(f).transpose(1, 0, 2).reshape(DM, WCOL)).astype(bf)
        cf = np.zeros((128, CF_W), f)
        cf[:, 0] = b_Q[hs].astype(f).reshape(WCOL)[:128]
        cf[:, 1] = b_Q[hs].astype(f).reshape(WCOL)[128:]
        cf[:, 2] = b_K[hs].astype(f).reshape(WCOL)[:128]
        cf[:, 3] = b_K[hs].astype(f).reshape(WCOL)[128:]
        cf[:, CF_BOB:CF_MASK] = b_O.astype(f)[None, :]
        cf[:, CF_MASK:CF_W] = (
            additive_attention_mask[b, 0, 0].astype(f).reshape(S_T, 128).T)
        cb = np.zeros((128, CB_W), bf)
        for h in range(H_LOC):
            cb[:, (DH + 1) * h:(DH + 1) * h + DH] = b_V[H_LOC * rk + h].astype(f)
            cb[:, (DH + 1) * h + DH] = 1.0
        cb[:, CB_TRIB:CB_W] = trib_host
        in_maps.append({
            "xt_q": np.ascontiguousarray(query_input[b].astype(f).T).astype(bf),
            "xt_k": np.ascontiguousarray(key_input[b].astype(f).T).astype(bf),
            "xt_v": np.ascontiguousarray(value_input[b].astype(f).T).astype(bf),
            "w_q": wq, "w_k": wk, "w_v": wv, "w_o": wo,
            "cf32": cf, "cbf16": cb,
        })
    return in_maps


def assemble_output(results):
    out = np.empty((B, S, DM), np.float32)
    for c in range(N_CORES):
        out[0, 256 * c:256 * (c + 1), :] = results[c]["out"][:256].astype(np.float32)
        out[1, 256 * c:256 * (c + 1), :] = results[c]["out"][256:].astype(np.float32)
    return out


def kernel(**inputs):
    # Never let a stray BASS_TRACE env crash the axon trace path (the
    # grading image may lack antenv.axon_hooks).
    os.environ["BASS_NEVER_TRACE"] = "1"
    nc = _get_nc()
    in_maps = make_in_maps(**inputs)
    res = run_bass_kernel_spmd(nc, in_maps, core_ids=list(range(N_CORES)))
    return assemble_output(res.results)
